# revision 1
# baseline (speedup 1.0000x reference)
"""Trainium2 Bass kernel for DiffusionHybridMoEBlock (8 NeuronCores).

Sharding: tokens (B*T=4096) split into 8 shards of 512 (one per core);
attention kv/ksum statistics AllReduce'd between the two cores sharing a
batch row. Expert MLPs computed densely per-core with combine weights folded
into pre-W2 scaling; cross-expert sum accumulated on-chip.

Layout: activations live "transposed" on device (feature on partitions,
tokens on free dim) so every matmul contracts over the partition dim with no
on-device transposes of big tensors. Host feeds q/k/v pre-transposed and
un-transposes the output (layout staging only, no host math on activations).

Precision: attention + LN statistics in float32r (full-speed fp32 PE mode),
router logits in true fp32, expert MLPs in bf16 with fp32 accumulation.
"""

import numpy as np
import ml_dtypes
from contextlib import ExitStack

import concourse.bass as bass
import concourse.tile as tile
import concourse.mybir as mybir
from concourse import bacc
from concourse.alu_op_type import AluOpType
from concourse.bass_utils import run_bass_kernel_spmd
from concourse.masks import make_identity

F32 = mybir.dt.float32
F32R = mybir.dt.float32r
BF16 = mybir.dt.bfloat16
AF = mybir.ActivationFunctionType
AX = mybir.AxisListType

B, T, D = 4, 1024, 1024
H, DH = 8, 128
E, FF = 8, 2048
LN_EPS = 1e-5
NCORES = 8
TOK = (B * T) // NCORES        # 512 tokens per core
P = 128
DT = D // P                    # 8 d-tiles
TT = TOK // P                  # 4 token-tiles
FT = FF // P                   # 16 f-tiles




DEBUG_OUTPUTS = False


def build_nc():
    nc = bacc.Bacc("TRN2", target_bir_lowering=False, debug=False,
                   num_devices=NCORES)

    io = {}
    def inp(nm, shape, dt):
        io[nm] = nc.dram_tensor(nm, shape, dt, kind="ExternalInput").ap()
    inp("qT", [D, TOK], F32R)
    inp("kT", [D, TOK], F32R)
    inp("vT", [D, TOK], F32R)
    inp("noiseT", [E, TOK], F32)
    inp("Wq", [D, D], F32R)
    inp("Wk", [D, D], F32R)
    inp("Wv", [D, D], F32R)
    inp("Wo", [D, D], F32R)
    inp("Wr", [D, E], F32)
    inp("W1b", [E, D, FF], BF16)
    inp("W2b", [E, FF, D], BF16)
    inp("lncols", [P, 10 * DT], F32)  # g/b pairs for v,k,q,moe,out
    inp("bocol", [P, DT], F32)
    inp("brcol", [E, 1], F32)
    inp("b1col", [P, E * FT], F32)
    inp("b2mat", [E, D], F32R)
    inp("esel", [E, E * P], F32R)
    inp("onesm", [P, P], F32R)
    io["yT"] = nc.dram_tensor("yT", [D, TOK], F32, kind="ExternalOutput").ap()
    if DEBUG_OUTPUTS:
        io["dbg_qresT"] = nc.dram_tensor("dbg_qresT", [D, TOK], F32R,
                                         kind="ExternalOutput").ap()
        io["dbg_xmT"] = nc.dram_tensor("dbg_xmT", [D, TOK], F32,
                                       kind="ExternalOutput").ap()
        io["dbg_logitsT"] = nc.dram_tensor("dbg_logitsT", [E, TOK], F32,
                                           kind="ExternalOutput").ap()
        io["dbg_comb"] = nc.dram_tensor("dbg_comb", [P, TT * E], F32,
                                        kind="ExternalOutput").ap()
        io["dbg_moeT"] = nc.dram_tensor("dbg_moeT", [D, TOK], F32,
                                        kind="ExternalOutput").ap()
        io["dbg_qfT"] = nc.dram_tensor("dbg_qfT", [D, TOK], F32R,
                                       kind="ExternalOutput").ap()
        io["dbg_kf"] = nc.dram_tensor("dbg_kf", [TOK, D], F32R,
                                      kind="ExternalOutput").ap()
        io["dbg_vh"] = nc.dram_tensor("dbg_vh", [TOK, D], F32R,
                                      kind="ExternalOutput").ap()
        io["dbg_kv2"] = nc.dram_tensor("dbg_kv2", [P, H * (DH + 1)], F32R,
                                       kind="ExternalOutput").ap()
        io["dbg_outfT"] = nc.dram_tensor("dbg_outfT", [D, TOK], F32R,
                                         kind="ExternalOutput").ap()


    with tile.TileContext(nc) as tc:
        with ExitStack() as ctx, \
                nc.allow_low_precision(reason="fp32r attention by design"):
            _build(ctx, tc, io)
    nc.compile()
    return nc


def _build(ctx, tc, io):
    nc = tc.nc

    # ---- whole-kernel pools (PSUM: 4 + 2 + 2 = 8 banks) ----
    psA = ctx.enter_context(tc.tile_pool(name="psA", bufs=5, space="PSUM"))
    psB = ctx.enter_context(tc.tile_pool(name="psB", bufs=2, space="PSUM"))
    psC = ctx.enter_context(tc.tile_pool(name="psC", bufs=1, space="PSUM"))
    const = ctx.enter_context(tc.tile_pool(name="const", bufs=1))
    live = ctx.enter_context(tc.tile_pool(name="live", bufs=1))
    tmp = ctx.enter_context(tc.tile_pool(name="tmp", bufs=4))
    rows = ctx.enter_context(tc.tile_pool(name="rows", bufs=2))
    wst = ctx.enter_context(tc.tile_pool(name="wst", bufs=3))
    dram = ctx.enter_context(tc.tile_pool(name="dram", bufs=1, space="DRAM"))

    def pa():
        return psA.tile([P, TOK], F32, space="PSUM", tag="a", name="pa")

    def pb():
        return psB.tile([P, TOK], F32, space="PSUM", tag="b", name="pb")

    def pc(shape):
        return psC.tile(shape, F32, space="PSUM", tag="c", name="pc")

    def t2k(dt=F32):
        return tmp.tile([P, TOK], dt, tag="t2k", name="t2k")

    def tsm(shape, dt=F32):
        return tmp.tile(shape, dt, tag="small", bufs=8, name="tsm")

    # ---- constants ----
    onesm = const.tile([P, P], F32R, name="onesm")
    nc.sync.dma_start(onesm[:], io["onesm"][:])
    ones = onesm[:, 0:1]
    onesr = onesm[0:1, :]
    ident = const.tile([P, P], F32, name="ident")
    make_identity(nc, ident[:])
    epsc = const.tile([P, 1], F32, name="epsc")
    nc.vector.memset(epsc[:], LN_EPS)

    lnc = const.tile([P, 10 * DT], F32, name="lnc")
    nc.sync.dma_start(lnc[:], io["lncols"][:])
    LN = {nm: (lnc[:, (2 * i) * DT:(2 * i + 1) * DT],
               lnc[:, (2 * i + 1) * DT:(2 * i + 2) * DT])
          for i, nm in enumerate(["v", "k", "q", "moe", "out"])}
    bo_c = const.tile([P, DT], F32, name="bo_c")
    nc.sync.dma_start(bo_c[:], io["bocol"][:])
    br_c = const.tile([E, 1], F32, name="br_c")
    nc.sync.dma_start(br_c[:], io["brcol"][:])
    b1_c = const.tile([P, E * FT], F32, name="b1_c")
    nc.sync.dma_start(b1_c[:], io["b1col"][:])
    b2_sb = const.tile([E, D], F32R, name="b2_sb")
    nc.sync.dma_start(b2_sb[:], io["b2mat"][:])
    noise_sb = const.tile([E, TOK], F32, name="noise_sb")
    nc.sync.dma_start(noise_sb[:], io["noiseT"][:])
    esel = const.tile([E, E * P], F32R, name="esel")
    nc.sync.dma_start(esel[:], io["esel"][:])
    wr_sb = const.tile([P, DT * E], F32, name="wr_sb")
    for d in range(DT):
        nc.sync.dma_start(wr_sb[:, d * E:(d + 1) * E],
                          io["Wr"][d * P:(d + 1) * P, :])


    # ---- transposed-layout LN ----
    # load_fn(d, pass_idx) -> [128, TOK] fp32 AP of x^T d-tile
    # emit(d, make_out) with make_out() -> dest AP
    def ln_t(load_fn, gb, emit, out_dtype=F32):
        g_c, b_c = gb
        ps_sum = pc([1, TOK])
        ps_sq = psB.tile([1, TOK], F32, space="PSUM", tag="b", name="ps_sq")
        for d in range(DT):
            x = load_fn(d, 0)
            sq = t2k(F32R)
            nc.scalar.activation(sq[:], x, AF.Square)
            nc.tensor.matmul(ps_sum[:], lhsT=ones, rhs=x,
                             start=(d == 0), stop=(d == DT - 1))
            nc.tensor.matmul(ps_sq[:], lhsT=ones, rhs=sq[:],
                             start=(d == 0), stop=(d == DT - 1))
        mean = rows.tile([1, TOK], F32, tag="mean", name="mean")[:]
        m2var = rows.tile([1, TOK], F32, tag="m2var", name="m2var")[:]
        sd = rows.tile([1, TOK], F32, tag="sd", name="sd")[:]
        r = rows.tile([1, TOK], F32R, tag="rrow", name="rrow")[:]
        nc.vector.tensor_scalar_mul(mean, ps_sum[:], 1.0 / D)
        nc.vector.tensor_tensor(m2var, mean, mean, op=AluOpType.mult)
        nc.vector.scalar_tensor_tensor(m2var, in0=ps_sq[:], scalar=1.0 / D,
                                       in1=m2var, op0=AluOpType.mult,
                                       op1=AluOpType.subtract)
        nc.scalar.activation(sd, m2var, AF.Sqrt, bias=epsc[0:1, 0:1])
        nc.vector.reciprocal(r, sd)
        nmr = rows.tile([1, TOK], F32R, tag="nmrrow", name="nmrrow")[:]
        nc.vector.scalar_tensor_tensor(nmr[:], in0=mean, scalar=-1.0,
                                       in1=r, op0=AluOpType.mult,
                                       op1=AluOpType.mult)
        rb = pb()
        nc.tensor.matmul(rb[:], lhsT=onesr, rhs=r,
                         start=True, stop=True)
        nmrb = pb()
        nc.tensor.matmul(nmrb[:], lhsT=onesr, rhs=nmr,
                         start=True, stop=True)
        for d in range(DT):
            x = load_fn(d, 1)
            a = t2k()
            nc.vector.scalar_tensor_tensor(a[:], in0=x,
                                           scalar=g_c[:, d:d + 1], in1=rb[:],
                                           op0=AluOpType.mult,
                                           op1=AluOpType.mult)
            c = t2k()
            nc.vector.tensor_scalar(c[:], in0=nmrb[:],
                                    scalar1=g_c[:, d:d + 1],
                                    scalar2=b_c[:, d:d + 1],
                                    op0=AluOpType.mult, op1=AluOpType.add)
            emit(d, a, c)

    def ln_cached(xtiles, gb, pool, nm, out_dtype=F32):
        outs = []
        def emit(d, a, c):
            o = pool.tile([P, TOK], out_dtype, name=f"{nm}{d}")
            nc.vector.tensor_add(o[:], a[:], c[:])
            outs.append(o)
        ln_t(lambda d, p: xtiles[d][:], gb, emit, out_dtype)
        return outs

    # streamed LN: x^T DMA'd twice from DRAM, x~ tiles cached in `pool`
    def ln_streamed(src, gb, pool, nm, out_dtype=F32):
        cache = {}
        def load(d, p):
            t = t2k(F32R)
            nc.sync.dma_start(t[:], src[d * P:(d + 1) * P, :])
            return t[:]
        outs = []
        def emit(d, a, c):
            o = pool.tile([P, TOK], out_dtype, name=f"{nm}{d}")
            nc.vector.tensor_add(o[:], a[:], c[:])
            outs.append(o)
        ln_t(load, gb, emit)
        return outs

    # ---- elu(x)+1 = exp(min(x,0)) + max(x,0) from a PSUM tile ----
    def elu1(dst, psum):
        m = t2k()
        nc.vector.tensor_scalar_min(m[:], psum[:], 0.0)
        ex = t2k()
        nc.scalar.activation(ex[:], m[:], AF.Exp)
        nc.vector.scalar_tensor_tensor(dst, in0=psum[:], scalar=0.0,
                                       in1=ex[:], op0=AluOpType.max,
                                       op1=AluOpType.add)

    # =================== attention ===================
    with tc.tile_pool(name="attn", bufs=1) as attn:
        # --- k/v projections (activation-stationary) -> natural [t, he] ---
        def proj_nat(xt, wsrc, outs, do_elu):
            for th in range(2):
                pss = {}
                for d in range(DT):
                    for jh in range(2):
                        wt = wst.tile([P, TOK], F32R, tag="wf", name="wt")
                        nc.sync.dma_start(
                            wt[:], wsrc[d * P:(d + 1) * P,
                                        jh * TOK:(jh + 1) * TOK])
                        for t in range(2):
                            if d == 0 and t == 0:
                                pass
                            if (t, jh) not in pss:
                                pss[(t, jh)] = pa()
                            tt_ = th * 2 + t
                            nc.tensor.matmul(
                                pss[(t, jh)][:],
                                lhsT=xt[d][:, tt_ * P:(tt_ + 1) * P],
                                rhs=wt[:],
                                start=(d == 0), stop=(d == DT - 1))
                for t in range(2):
                    for jh in range(2):
                        dst = outs[th * 2 + t][:, jh * TOK:(jh + 1) * TOK]
                        if do_elu:
                            elu1(dst, pss[(t, jh)])
                        else:
                            nc.scalar.activation(dst, pss[(t, jh)][:], AF.Copy)

        kf = [attn.tile([P, D], F32R, name=f"kf{t}") for t in range(TT)]
        with tc.tile_pool(name="pxk", bufs=1) as pxk:
            xk = ln_streamed(io["kT"], LN["k"], pxk, "xk", out_dtype=F32R)
            proj_nat(xk, io["Wk"], kf, True)
        vh = [attn.tile([P, D], F32R, name=f"vh{t}") for t in range(TT)]
        with tc.tile_pool(name="pxv", bufs=1) as pxv:
            xv = ln_streamed(io["vT"], LN["v"], pxv, "xv", out_dtype=F32R)
            proj_nat(xv, io["Wv"], vh, False)

        # --- kv[h] (+ksum col) over local tokens; AllReduce over pair ---
        kv_sb = tmp.tile([P, H * (DH + 1)], F32R, tag="kvsb", bufs=1, name="kv_sb")
        for h in range(H):
            ps = pc([P, DH + 2])
            for t in range(TT):
                nc.tensor.matmul(ps[:, :DH],
                                 lhsT=kf[t][:, h * DH:(h + 1) * DH],
                                 rhs=vh[t][:, h * DH:(h + 1) * DH],
                                 start=(t == 0), stop=(t == TT - 1))
            for t in range(TT):
                nc.tensor.matmul(ps[:, DH:DH + 2],
                                 lhsT=kf[t][:, h * DH:(h + 1) * DH],
                                 rhs=onesm[:, 0:2],
                                 start=(t == 0), stop=(t == TT - 1))
            nc.scalar.activation(kv_sb[:, h * (DH + 1):(h + 1) * (DH + 1)],
                                 ps[:, :DH + 1], AF.Copy)

        cc_in = dram.tile([P, H * (DH + 1)], F32R, name="cc_in")
        cc_out = dram.tile([P, H * (DH + 1)], F32R, name="cc_out")
        nc.sync.dma_start(cc_in[:], kv_sb[:])
        nc.gpsimd.collective_compute(
            "AllReduce", AluOpType.add,
            replica_groups=[[0, 1], [2, 3], [4, 5], [6, 7]],
            ins=[cc_in[:]], outs=[cc_out[:]])
        qf = [attn.tile([P, TOK], F32R, name=f"qf{j}") for j in range(DT)]
        # --- q projection (weight-stationary): qh^T -> elu -> qf ---
        with tc.tile_pool(name="pxq", bufs=1) as pxq:
            xq = ln_streamed(io["qT"], LN["q"], pxq, "xq", out_dtype=F32R)
            for jh in range(2):
                pss = []
                for d in range(DT):
                    wt = wst.tile([P, 4 * P], F32R, tag="wf", name="wt")
                    nc.sync.dma_start(
                        wt[:], io["Wq"][d * P:(d + 1) * P,
                                        jh * 4 * P:(jh + 1) * 4 * P])
                    for j in range(4):
                        if d == 0:
                            pss.append(pa())
                        nc.tensor.matmul(pss[j][:],
                                         lhsT=wt[:, j * P:(j + 1) * P],
                                         rhs=xq[d][:],
                                         start=(d == 0), stop=(d == DT - 1))
                for j in range(4):
                    elu1(qf[jh * 4 + j][:], pss[j])


        kv2 = attn.tile([P, H * (DH + 1)], F32R, name="kv2")
        nc.sync.dma_start(kv2[:], cc_out[:])
        for h in range(H):
            ks = kv2[:, h * (DH + 1) + DH:h * (DH + 1) + DH + 1]
            nc.vector.tensor_scalar_add(ks, ks, 1e-6)

        # --- out[h] = qf_h @ kv_h ; denom ; divide ---
        outf = [attn.tile([P, TOK], F32R, name=f"outf{h}") for h in range(H)]
        for h in range(H):
            kvh = kv2[:, h * (DH + 1):h * (DH + 1) + DH]
            ksh = kv2[:, h * (DH + 1) + DH:h * (DH + 1) + DH + 1]
            pso = pa()
            nc.tensor.matmul(pso[:], lhsT=kvh, rhs=qf[h][:],
                             start=True, stop=True)
            psd = pc([1, TOK])
            nc.tensor.matmul(psd[:], lhsT=ksh, rhs=qf[h][:],
                             start=True, stop=True)
            rdh = rows.tile([1, TOK], F32R, tag="rdh", name="rdh")
            nc.vector.reciprocal(rdh[:], psd[:])
            rb = pb()
            nc.tensor.matmul(rb[:], lhsT=onesr,
                             rhs=rdh[:], start=True, stop=True)
            nc.scalar.activation(outf[h][:], pso[:], AF.Copy)
            nc.vector.tensor_mul(outf[h][:], outf[h][:], rb[:])

        if DEBUG_OUTPUTS:
            for j in range(DT):
                nc.sync.dma_start(io["dbg_qfT"][j * P:(j + 1) * P, :], qf[j][:])
                nc.sync.dma_start(io["dbg_outfT"][j * P:(j + 1) * P, :],
                                  outf[j][:])
            for t in range(TT):
                nc.sync.dma_start(io["dbg_kf"][t * P:(t + 1) * P, :], kf[t][:])
                nc.sync.dma_start(io["dbg_vh"][t * P:(t + 1) * P, :], vh[t][:])
            nc.sync.dma_start(io["dbg_kv2"][:], kv2[:])

        # --- attn^T = Wo-stationary @ outf ; q_res = q + attn + bo ---
        qres = [live.tile([P, TOK], F32R, name=f"qres{j}") for j in range(DT)]
        for jh in range(2):
            pss = []
            for e in range(DT):
                wt = wst.tile([P, 4 * P], F32R, tag="wf", name="wt")
                nc.sync.dma_start(
                    wt[:], io["Wo"][e * P:(e + 1) * P,
                                    jh * 4 * P:(jh + 1) * 4 * P])
                for j in range(4):
                    if e == 0:
                        pss.append(pa())
                    nc.tensor.matmul(pss[j][:],
                                     lhsT=wt[:, j * P:(j + 1) * P],
                                     rhs=outf[e][:],
                                     start=(e == 0), stop=(e == DT - 1))
            for j in range(4):
                dj = jh * 4 + j
                qt = t2k(F32R)
                nc.sync.dma_start(qt[:], io["qT"][dj * P:(dj + 1) * P, :])
                nc.vector.scalar_tensor_tensor(
                    qres[dj][:], in0=pss[j][:], scalar=bo_c[:, dj:dj + 1],
                    in1=qt[:], op0=AluOpType.add, op1=AluOpType.add)

    if DEBUG_OUTPUTS:
        for d in range(DT):
            nc.sync.dma_start(io["dbg_qresT"][d * P:(d + 1) * P, :],
                              qres[d][:])

    # =================== router ===================
    xm = ln_cached(qres, LN["moe"], live, "xm")
    if DEBUG_OUTPUTS:
        for d in range(DT):
            nc.sync.dma_start(io["dbg_xmT"][d * P:(d + 1) * P, :], xm[d][:])

    psl = pc([E, TOK])
    for d in range(DT):
        nc.tensor.matmul(psl[:], lhsT=wr_sb[:, d * E:(d + 1) * E],
                         rhs=xm[d][:], start=(d == 0), stop=(d == DT - 1))
    logitsT = tmp.tile([E, TOK], F32, tag="lgt", bufs=1, name="logitsT")
    nc.vector.scalar_tensor_tensor(logitsT[:], in0=noise_sb[:], scalar=0.1,
                                   in1=psl[:], op0=AluOpType.mult,
                                   op1=AluOpType.add)
    nc.vector.tensor_scalar_add(logitsT[:], logitsT[:], br_c[:])

    comb = tmp.tile([P, TT * E], F32, tag="comb", bufs=1, name="comb")
    for t in range(TT):
        pst = pc([P, E])
        nc.tensor.transpose(pst[:], logitsT[:, t * P:(t + 1) * P],
                            ident[:E, :E])
        lg = tsm([P, E])
        nc.vector.tensor_copy(lg[:], pst[:])
        nmx = tsm([P, 1])
        nc.vector.reduce_max(nmx[:], lg[:], axis=AX.X, negate=True)
        ex = tsm([P, E])
        ssum = tsm([P, 1])
        nc.scalar.activation(ex[:], lg[:], AF.Exp, bias=nmx[:],
                             accum_out=ssum[:])
        rs = tsm([P, 1])
        nc.vector.reciprocal(rs[:], ssum[:])
        sc = tsm([P, E])
        nc.vector.tensor_scalar_mul(sc[:], ex[:], rs[:])
        m1v = tsm([P, 1])
        nc.vector.reduce_max(m1v[:], lg[:], axis=AX.X)
        m1 = tsm([P, E])
        nc.vector.tensor_scalar(m1[:], in0=lg[:], scalar1=m1v[:],
                                scalar2=None, op0=AluOpType.is_ge)
        lg2 = tsm([P, E])
        nc.vector.scalar_tensor_tensor(lg2[:], in0=m1[:], scalar=-1e30,
                                       in1=lg[:], op0=AluOpType.mult,
                                       op1=AluOpType.add)
        m2v = tsm([P, 1])
        nc.vector.reduce_max(m2v[:], lg2[:], axis=AX.X)
        msk = tsm([P, E])
        nc.vector.tensor_scalar(msk[:], in0=lg2[:], scalar1=m2v[:],
                                scalar2=None, op0=AluOpType.is_ge)
        nc.vector.tensor_add(msk[:], msk[:], m1[:])
        nc.vector.tensor_mul(comb[:, t * E:(t + 1) * E], msk[:], sc[:])

    if DEBUG_OUTPUTS:
        nc.sync.dma_start(io["dbg_logitsT"][:], logitsT[:])
        nc.sync.dma_start(io["dbg_comb"][:], comb[:])
    wT = live.tile([E, TOK], F32R, name="wT")
    for t in range(TT):
        pst = pc([E, P])
        nc.tensor.transpose(pst[:], comb[:, t * E:(t + 1) * E], ident[:])
        nc.scalar.activation(wT[:, t * P:(t + 1) * P], pst[:], AF.Copy)

    # =================== experts (bf16) ===================
    moe = [live.tile([P, TOK], F32, name=f"moe{d}") for d in range(DT)]
    for dj in range(DT):  # init with sum_e w_e * b2_e
        ps = pa()
        nc.tensor.matmul(ps[:], lhsT=b2_sb[:, dj * P:(dj + 1) * P],
                         rhs=wT[:], start=True, stop=True)
        nc.scalar.activation(moe[dj][:], ps[:], AF.Copy)

    with tc.tile_pool(name="exp", bufs=1) as expp:
        xmb = []
        for d in range(DT):
            t = expp.tile([P, TOK], BF16, name=f"xmb{d}")
            nc.vector.tensor_copy(t[:], xm[d][:])
            xmb.append(t)
        hsc = [expp.tile([P, TOK], BF16, name=f"hsc{f}") for f in range(FT)]
        for e in range(E):
            wb = pb()
            nc.tensor.matmul(wb[:], lhsT=esel[:, e * P:(e + 1) * P],
                             rhs=wT[:], start=True, stop=True)
            for fs in range(FT // 2):  # 256-col f-slabs
                w1t = []
                for d in range(DT):
                    wt = wst.tile([P, 256], BF16, tag="w1", bufs=11, name="w1t")
                    nc.sync.dma_start(
                        wt[:], io["W1b"][e, d * P:(d + 1) * P,
                                         fs * 256:(fs + 1) * 256])
                    w1t.append(wt)
                for fo in range(2):
                    f = fs * 2 + fo
                    ps = pa()
                    for d in range(DT):
                        nc.tensor.matmul(ps[:],
                                         lhsT=w1t[d][:, fo * P:(fo + 1) * P],
                                         rhs=xmb[d][:],
                                         start=(d == 0), stop=(d == DT - 1))
                    hg = t2k()
                    nc.scalar.activation(
                        hg[:], ps[:], AF.Gelu,
                        bias=b1_c[:, e * FT + f:e * FT + f + 1])
                    nc.vector.tensor_mul(hsc[f][:], hg[:], wb[:])
            for ds in range(DT // 2):  # 256-col d'-slabs
                w2t = []
                for f in range(FT):
                    wt = wst.tile([P, 256], BF16, tag="w2", bufs=22, name="w2t")
                    nc.sync.dma_start(
                        wt[:], io["W2b"][e, f * P:(f + 1) * P,
                                         ds * 256:(ds + 1) * 256])
                    w2t.append(wt)
                for do in range(2):
                    dj = ds * 2 + do
                    ps = pa()
                    for f in range(FT):
                        nc.tensor.matmul(ps[:],
                                         lhsT=w2t[f][:, do * P:(do + 1) * P],
                                         rhs=hsc[f][:],
                                         start=(f == 0), stop=(f == FT - 1))
                    nc.vector.tensor_add(moe[dj][:], moe[dj][:], ps[:])

    if DEBUG_OUTPUTS:
        for d in range(DT):
            nc.sync.dma_start(io["dbg_moeT"][d * P:(d + 1) * P, :],
                              moe[d][:])

    # =================== tail: LN + residual ===================
    with tc.tile_pool(name="tail", bufs=1) as tail:
        u = []
        for d in range(DT):
            t = tail.tile([P, TOK], F32R, name=f"u{d}")
            nc.vector.tensor_add(t[:], moe[d][:], xm[d][:])
            u.append(t)

        def emit(d, a, c):
            mo = t2k()
            nc.vector.tensor_add(mo[:], a[:], c[:])
            yt = t2k()
            nc.vector.tensor_add(yt[:], qres[d][:], mo[:])
            nc.sync.dma_start(io["yT"][d * P:(d + 1) * P, :], yt[:])
        ln_t(lambda d, p: u[d][:], LN["out"], emit)


_NC_CACHE = None
_LAST_RES = None


def _get_nc():
    global _NC_CACHE
    if _NC_CACHE is None:
        _NC_CACHE = build_nc()
    return _NC_CACHE


def _bf16(a):
    """fast float32 -> bfloat16 round-to-nearest-even"""
    a = np.ascontiguousarray(a, dtype=np.float32)
    u = a.view(np.uint32)
    r = (u + 0x7FFF + ((u >> 16) & 1)) >> 16
    return np.ascontiguousarray(r.astype(np.uint16)).view(ml_dtypes.bfloat16)


def kernel(v, k, q, noise, g_v, b_v, g_k, b_k, g_q, b_q, g_moe, b_moe,
           g_out, b_out, Wq, Wk, Wv, Wo, bo, Wr, br, W1, b1, W2, b2, top_k):
    assert int(top_k) == 2
    nc = _get_nc()
    f32 = np.float32

    def col(x):  # [D] -> [128, DT] per-d-tile columns
        return np.ascontiguousarray(np.asarray(x, f32).reshape(-1, P).T)

    lncols = np.concatenate(
        [col(g_v), col(b_v), col(g_k), col(b_k), col(g_q), col(b_q),
         col(g_moe), col(b_moe), col(g_out), col(b_out)], axis=1)
    shared = {
        "Wq": np.asarray(Wq, f32), "Wk": np.asarray(Wk, f32),
        "Wv": np.asarray(Wv, f32), "Wo": np.asarray(Wo, f32),
        "Wr": np.asarray(Wr, f32),
        "W1b": _bf16(W1), "W2b": _bf16(W2),
        "lncols": np.ascontiguousarray(lncols), "bocol": col(bo),
        "brcol": np.ascontiguousarray(np.asarray(br, f32).reshape(E, 1)),
        "b1col": col(np.asarray(b1, f32).reshape(-1)),
        "b2mat": np.asarray(b2, f32),
        "esel": np.ascontiguousarray(
            np.repeat(np.eye(E, dtype=f32), P, axis=1).reshape(E, E * P)),
        "onesm": np.ones((P, P), f32),
    }
    qf_ = np.asarray(q, f32).reshape(B * T, D)
    kf_ = np.asarray(k, f32).reshape(B * T, D)
    vf_ = np.asarray(v, f32).reshape(B * T, D)
    nf_ = np.asarray(noise, f32)
    in_maps = []
    for c in range(NCORES):
        sl = slice(c * TOK, (c + 1) * TOK)
        m = dict(shared)
        m["qT"] = np.ascontiguousarray(qf_[sl].T)
        m["kT"] = np.ascontiguousarray(kf_[sl].T)
        m["vT"] = np.ascontiguousarray(vf_[sl].T)
        m["noiseT"] = np.ascontiguousarray(nf_[sl].T)
        in_maps.append(m)

    res = run_bass_kernel_spmd(nc, in_maps, core_ids=list(range(NCORES)))
    global _LAST_RES
    _LAST_RES = res
    out = np.empty((B * T, D), f32)
    for c in range(NCORES):
        out[c * TOK:(c + 1) * TOK] = res.results[c]["yT"].T
    return out.reshape(B, T, D)


if __name__ == "__main__":
    build_nc()
    print("build ok")



# revision 16
# speedup vs baseline: 1.5501x; 1.5501x over previous
"""Trainium2 Bass kernel for DiffusionHybridMoEBlock (8 NeuronCores).

Sharding: tokens (B*T=4096) split into 8 shards of 512 (one per core);
attention kv/ksum statistics AllReduce'd between the two cores sharing a
batch row. Expert MLPs computed densely per-core with combine weights folded
into pre-W2 scaling; cross-expert sum accumulated on-chip.

Layout: activations live "transposed" on device (feature on partitions,
tokens on free dim) so every matmul contracts over the partition dim with no
on-device transposes of big tensors. Host feeds q/k/v pre-transposed and
un-transposes the output (layout staging only, no host math on activations).

Precision: attention + LN statistics in float32r (full-speed fp32 PE mode),
router logits in true fp32, expert MLPs in bf16 with fp32 accumulation.
"""

import numpy as np
import ml_dtypes
from contextlib import ExitStack

import concourse.bass as bass
import concourse.tile as tile
import concourse.mybir as mybir
from concourse import bacc
from concourse.alu_op_type import AluOpType
from concourse.bass_utils import run_bass_kernel_spmd
from concourse.masks import make_identity

F32 = mybir.dt.float32
F32R = mybir.dt.float32r
BF16 = mybir.dt.bfloat16
FP8 = mybir.dt.float8e4
AF = mybir.ActivationFunctionType
AX = mybir.AxisListType

B, T, D = 4, 1024, 1024
H, DH = 8, 128
E, FF = 8, 2048
LN_EPS = 1e-5
NCORES = 8
TOK = (B * T) // NCORES        # 512 tokens per core
P = 128
DT = D // P                    # 8 d-tiles
TT = TOK // P                  # 4 token-tiles
FT = FF // P                   # 16 f-tiles
CAP = 160                      # per-(core,expert) token capacity
WSC = 64.0                     # fp8 weight pre-scale




DEBUG_OUTPUTS = False


def build_nc():
    nc = bacc.Bacc("TRN2", target_bir_lowering=False, debug=False,
                   num_devices=NCORES)

    io = {}
    def inp(nm, shape, dt):
        io[nm] = nc.dram_tensor(nm, shape, dt, kind="ExternalInput").ap()
    inp("qT", [D, TOK], F32R)
    inp("kT", [D, TOK], F32R)
    inp("vT", [D, TOK], F32R)
    inp("noiseT", [E, TOK], F32)
    inp("Wq", [D, D], F32R)
    inp("Wk", [D, D], F32R)
    inp("Wv", [D, D], F32R)
    inp("Wo", [D, D], F32R)
    inp("Wr", [D, E], F32)
    inp("W1q", [E, P, DT * FF], FP8)    # packed: [e, p, d*FF+f] = 64*W1[e,d*128+p,f]
    inp("W2q", [E, P, FT * D], FP8)     # packed: [e, p, k*D+j] = 64*W2[e,k*128+p,j]
    inp("lncols", [P, 10 * DT], F32)  # g/b pairs for v,k,q,moe,out
    inp("bocol", [P, DT], F32)
    inp("brcol", [E, 1], F32)
    inp("b1col", [P, E * FT], F32)
    inp("b2mat", [E, D], F32R)
    inp("esel", [E, E * P], F32R)
    inp("onesm", [P, P], F32R)
    inp("lmat", [P, P + 1], F32)     # strict lower-tri ones + ones col
    inp("iotac", [P, CAP], F32)       # row 0..CAP-1 on every partition
    inp("ccols", [P, 2], F32)         # col0 = p, col1 = 128+p
    io["yT"] = nc.dram_tensor("yT", [D, TOK], F32, kind="ExternalOutput").ap()
    if DEBUG_OUTPUTS:
        io["dbg_qresT"] = nc.dram_tensor("dbg_qresT", [D, TOK], F32R,
                                         kind="ExternalOutput").ap()
        io["dbg_xmT"] = nc.dram_tensor("dbg_xmT", [D, TOK], F32,
                                       kind="ExternalOutput").ap()
        io["dbg_logitsT"] = nc.dram_tensor("dbg_logitsT", [E, TOK], F32,
                                           kind="ExternalOutput").ap()
        io["dbg_comb"] = nc.dram_tensor("dbg_comb", [P, TT * E], F32,
                                        kind="ExternalOutput").ap()
        io["dbg_moeT"] = nc.dram_tensor("dbg_moeT", [D, TOK], F32,
                                        kind="ExternalOutput").ap()
        io["dbg_qfT"] = nc.dram_tensor("dbg_qfT", [D, TOK], F32R,
                                       kind="ExternalOutput").ap()
        io["dbg_kf"] = nc.dram_tensor("dbg_kf", [TOK, D], F32R,
                                      kind="ExternalOutput").ap()
        io["dbg_vh"] = nc.dram_tensor("dbg_vh", [TOK, D], F32R,
                                      kind="ExternalOutput").ap()
        io["dbg_kv2"] = nc.dram_tensor("dbg_kv2", [P, H * (DH + 1)], F32R,
                                       kind="ExternalOutput").ap()
        io["dbg_outfT"] = nc.dram_tensor("dbg_outfT", [D, TOK], F32R,
                                         kind="ExternalOutput").ap()


    with tile.TileContext(nc) as tc:
        with ExitStack() as ctx, \
                nc.allow_low_precision(reason="fp32r attention by design"):
            _build(ctx, tc, io)
    nc.compile()
    return nc


def _build(ctx, tc, io):
    nc = tc.nc

    # ---- whole-kernel pools (PSUM: 4 + 2 + 2 = 8 banks) ----
    psA = ctx.enter_context(tc.tile_pool(name="psA", bufs=5, space="PSUM"))
    psB = ctx.enter_context(tc.tile_pool(name="psB", bufs=2, space="PSUM"))
    psC = ctx.enter_context(tc.tile_pool(name="psC", bufs=1, space="PSUM"))
    const = ctx.enter_context(tc.tile_pool(name="const", bufs=1))
    live = ctx.enter_context(tc.tile_pool(name="live", bufs=1))
    tmp = ctx.enter_context(tc.tile_pool(name="tmp", bufs=4))
    rows = ctx.enter_context(tc.tile_pool(name="rows", bufs=2))
    wst = ctx.enter_context(tc.tile_pool(name="wst", bufs=3))
    dram = ctx.enter_context(tc.tile_pool(name="dram", bufs=1, space="DRAM"))

    def pa():
        return psA.tile([P, TOK], F32, space="PSUM", tag="a", name="pa")

    def pb():
        return psB.tile([P, TOK], F32, space="PSUM", tag="b", name="pb")

    def pc(shape):
        return psC.tile(shape, F32, space="PSUM", tag="c", name="pc")

    def t2k(dt=F32):
        return tmp.tile([P, TOK], dt, tag="t2k", name="t2k")

    def tsm(shape, dt=F32):
        return tmp.tile(shape, dt, tag="small", bufs=6, name="tsm")

    # ---- constants ----
    onesm = const.tile([P, P], F32R, name="onesm")
    nc.sync.dma_start(onesm[:], io["onesm"][:])
    ones = onesm[:, 0:1]
    onesr = onesm[0:1, :]
    ident = const.tile([P, P], F32, name="ident")
    make_identity(nc, ident[:])
    epsc = const.tile([P, 1], F32, name="epsc")
    nc.vector.memset(epsc[:], LN_EPS)

    lnc = const.tile([P, 10 * DT], F32, name="lnc")
    nc.sync.dma_start(lnc[:], io["lncols"][:])
    LN = {nm: (lnc[:, (2 * i) * DT:(2 * i + 1) * DT],
               lnc[:, (2 * i + 1) * DT:(2 * i + 2) * DT])
          for i, nm in enumerate(["v", "k", "q", "moe", "out"])}
    bo_c = const.tile([P, DT], F32, name="bo_c")
    nc.sync.dma_start(bo_c[:], io["bocol"][:])
    br_c = const.tile([E, 1], F32, name="br_c")
    nc.sync.dma_start(br_c[:], io["brcol"][:])
    b1_c = const.tile([P, E * FT], F32, name="b1_c")
    nc.sync.dma_start(b1_c[:], io["b1col"][:])
    b2_sb = const.tile([E, D], F32R, name="b2_sb")
    nc.sync.dma_start(b2_sb[:], io["b2mat"][:])
    noise_sb = const.tile([E, TOK], F32, name="noise_sb")
    nc.sync.dma_start(noise_sb[:], io["noiseT"][:])
    esel = const.tile([E, E * P], F32R, name="esel")
    nc.sync.dma_start(esel[:], io["esel"][:])
    wr_sb = const.tile([P, DT * E], F32, name="wr_sb")
    for d in range(DT):
        nc.sync.dma_start(wr_sb[:, d * E:(d + 1) * E],
                          io["Wr"][d * P:(d + 1) * P, :])


    # ---- transposed-layout LN ----
    # load_fn(d, pass_idx) -> [128, TOK] fp32 AP of x^T d-tile
    # emit(d, make_out) with make_out() -> dest AP
    def ln_t(load_fn, gb, emit, out_dtype=F32):
        g_c, b_c = gb
        ps_sum = pc([1, TOK])
        ps_sq = psB.tile([1, TOK], F32, space="PSUM", tag="b", name="ps_sq")
        for d in range(DT):
            x = load_fn(d, 0)
            sq = t2k(F32R)
            nc.scalar.activation(sq[:], x, AF.Square)
            nc.tensor.matmul(ps_sum[:], lhsT=ones, rhs=x,
                             start=(d == 0), stop=(d == DT - 1))
            nc.tensor.matmul(ps_sq[:], lhsT=ones, rhs=sq[:],
                             start=(d == 0), stop=(d == DT - 1))
        mean = rows.tile([1, TOK], F32, tag="mean", name="mean")[:]
        m2var = rows.tile([1, TOK], F32, tag="m2var", name="m2var")[:]
        sd = rows.tile([1, TOK], F32, tag="sd", name="sd")[:]
        r = rows.tile([1, TOK], F32R, tag="rrow", name="rrow")[:]
        nc.vector.tensor_scalar_mul(mean, ps_sum[:], 1.0 / D)
        nc.vector.tensor_tensor(m2var, mean, mean, op=AluOpType.mult)
        nc.vector.scalar_tensor_tensor(m2var, in0=ps_sq[:], scalar=1.0 / D,
                                       in1=m2var, op0=AluOpType.mult,
                                       op1=AluOpType.subtract)
        nc.scalar.activation(sd, m2var, AF.Sqrt, bias=epsc[0:1, 0:1])
        nc.vector.reciprocal(r, sd)
        nmr = rows.tile([1, TOK], F32R, tag="nmrrow", name="nmrrow")[:]
        nc.vector.scalar_tensor_tensor(nmr[:], in0=mean, scalar=-1.0,
                                       in1=r, op0=AluOpType.mult,
                                       op1=AluOpType.mult)
        rb = pb()
        nc.tensor.matmul(rb[:], lhsT=onesr, rhs=r,
                         start=True, stop=True)
        nmrb = pb()
        nc.tensor.matmul(nmrb[:], lhsT=onesr, rhs=nmr,
                         start=True, stop=True)
        for d in range(DT):
            x = load_fn(d, 1)
            a = t2k()
            nc.vector.scalar_tensor_tensor(a[:], in0=x,
                                           scalar=g_c[:, d:d + 1], in1=rb[:],
                                           op0=AluOpType.mult,
                                           op1=AluOpType.mult)
            c = t2k()
            nc.vector.tensor_scalar(c[:], in0=nmrb[:],
                                    scalar1=g_c[:, d:d + 1],
                                    scalar2=b_c[:, d:d + 1],
                                    op0=AluOpType.mult, op1=AluOpType.add)
            emit(d, a, c)

    def ln_cached(xtiles, gb, pool, nm, out_dtype=F32):
        outs = []
        def emit(d, a, c):
            o = pool.tile([P, TOK], out_dtype, name=f"{nm}{d}")
            nc.vector.tensor_add(o[:], a[:], c[:])
            outs.append(o)
        ln_t(lambda d, p: xtiles[d][:], gb, emit, out_dtype)
        return outs

    # streamed LN: x^T DMA'd twice from DRAM, x~ tiles cached in `pool`
    def ln_streamed(src, gb, pool, nm, out_dtype=F32):
        cache = {}
        def load(d, p):
            t = t2k(F32R)
            nc.sync.dma_start(t[:], src[d * P:(d + 1) * P, :])
            return t[:]
        outs = []
        def emit(d, a, c):
            o = pool.tile([P, TOK], out_dtype, name=f"{nm}{d}")
            nc.vector.tensor_add(o[:], a[:], c[:])
            outs.append(o)
        ln_t(load, gb, emit)
        return outs

    # ---- elu(x)+1 = exp(min(x,0)) + max(x,0) from a PSUM tile ----
    def elu1(dst, psum):
        m = t2k()
        nc.vector.tensor_scalar_min(m[:], psum[:], 0.0)
        ex = t2k()
        nc.scalar.activation(ex[:], m[:], AF.Exp)
        nc.vector.scalar_tensor_tensor(dst, in0=psum[:], scalar=0.0,
                                       in1=ex[:], op0=AluOpType.max,
                                       op1=AluOpType.add)

    # =================== attention ===================
    with tc.tile_pool(name="attn", bufs=1) as attn:
        # --- k/v projections (activation-stationary) -> natural [t, he] ---
        def proj_nat(xt, wsrc, outs, do_elu):
            for th in range(2):
                pss = {}
                for d in range(DT):
                    for jh in range(2):
                        wt = wst.tile([P, TOK], F32R, tag="wf", name="wt")
                        nc.sync.dma_start(
                            wt[:], wsrc[d * P:(d + 1) * P,
                                        jh * TOK:(jh + 1) * TOK])
                        for t in range(2):
                            if d == 0 and t == 0:
                                pass
                            if (t, jh) not in pss:
                                pss[(t, jh)] = pa()
                            tt_ = th * 2 + t
                            nc.tensor.matmul(
                                pss[(t, jh)][:],
                                lhsT=xt[d][:, tt_ * P:(tt_ + 1) * P],
                                rhs=wt[:],
                                start=(d == 0), stop=(d == DT - 1))
                for t in range(2):
                    for jh in range(2):
                        dst = outs[th * 2 + t][:, jh * TOK:(jh + 1) * TOK]
                        if do_elu:
                            elu1(dst, pss[(t, jh)])
                        else:
                            nc.scalar.activation(dst, pss[(t, jh)][:], AF.Copy)

        kf = [attn.tile([P, D], F32R, name=f"kf{t}") for t in range(TT)]
        with tc.tile_pool(name="pxk", bufs=1) as pxk:
            xk = ln_streamed(io["kT"], LN["k"], pxk, "xk", out_dtype=F32R)
            proj_nat(xk, io["Wk"], kf, True)
        vh = [attn.tile([P, D], F32R, name=f"vh{t}") for t in range(TT)]
        with tc.tile_pool(name="pxv", bufs=1) as pxv:
            xv = ln_streamed(io["vT"], LN["v"], pxv, "xv", out_dtype=F32R)
            proj_nat(xv, io["Wv"], vh, False)

        # --- kv[h] (+ksum col) over local tokens; AllReduce over pair ---
        kv_sb = tmp.tile([P, H * (DH + 1)], F32R, tag="kvsb", bufs=1, name="kv_sb")
        for h in range(H):
            ps = pc([P, DH + 2])
            for t in range(TT):
                nc.tensor.matmul(ps[:, :DH],
                                 lhsT=kf[t][:, h * DH:(h + 1) * DH],
                                 rhs=vh[t][:, h * DH:(h + 1) * DH],
                                 start=(t == 0), stop=(t == TT - 1))
            for t in range(TT):
                nc.tensor.matmul(ps[:, DH:DH + 2],
                                 lhsT=kf[t][:, h * DH:(h + 1) * DH],
                                 rhs=onesm[:, 0:2],
                                 start=(t == 0), stop=(t == TT - 1))
            nc.scalar.activation(kv_sb[:, h * (DH + 1):(h + 1) * (DH + 1)],
                                 ps[:, :DH + 1], AF.Copy)

        cc_in = dram.tile([P, H * (DH + 1)], F32R, name="cc_in")
        cc_out = dram.tile([P, H * (DH + 1)], F32R, name="cc_out")
        nc.sync.dma_start(cc_in[:], kv_sb[:])
        nc.gpsimd.collective_compute(
            "AllReduce", AluOpType.add,
            replica_groups=[[0, 1], [2, 3], [4, 5], [6, 7]],
            ins=[cc_in[:]], outs=[cc_out[:]])
        qf = [attn.tile([P, TOK], F32R, name=f"qf{j}") for j in range(DT)]
        # --- q projection (weight-stationary): qh^T -> elu -> qf ---
        with tc.tile_pool(name="pxq", bufs=1) as pxq:
            xq = ln_streamed(io["qT"], LN["q"], pxq, "xq", out_dtype=F32R)
            for jh in range(2):
                pss = []
                for d in range(DT):
                    wt = wst.tile([P, 4 * P], F32R, tag="wf", name="wt")
                    nc.sync.dma_start(
                        wt[:], io["Wq"][d * P:(d + 1) * P,
                                        jh * 4 * P:(jh + 1) * 4 * P])
                    for j in range(4):
                        if d == 0:
                            pss.append(pa())
                        nc.tensor.matmul(pss[j][:],
                                         lhsT=wt[:, j * P:(j + 1) * P],
                                         rhs=xq[d][:],
                                         start=(d == 0), stop=(d == DT - 1))
                for j in range(4):
                    elu1(qf[jh * 4 + j][:], pss[j])


        kv2 = attn.tile([P, H * (DH + 1)], F32R, name="kv2")
        nc.sync.dma_start(kv2[:], cc_out[:])
        for h in range(H):
            ks = kv2[:, h * (DH + 1) + DH:h * (DH + 1) + DH + 1]
            nc.vector.tensor_scalar_add(ks, ks, 1e-6)

        # --- out[h] = qf_h @ kv_h ; denom ; divide ---
        outf = [attn.tile([P, TOK], F32R, name=f"outf{h}") for h in range(H)]
        for h in range(H):
            kvh = kv2[:, h * (DH + 1):h * (DH + 1) + DH]
            ksh = kv2[:, h * (DH + 1) + DH:h * (DH + 1) + DH + 1]
            pso = pa()
            nc.tensor.matmul(pso[:], lhsT=kvh, rhs=qf[h][:],
                             start=True, stop=True)
            psd = pc([1, TOK])
            nc.tensor.matmul(psd[:], lhsT=ksh, rhs=qf[h][:],
                             start=True, stop=True)
            rdh = rows.tile([1, TOK], F32R, tag="rdh", name="rdh")
            nc.vector.reciprocal(rdh[:], psd[:])
            rb = pb()
            nc.tensor.matmul(rb[:], lhsT=onesr,
                             rhs=rdh[:], start=True, stop=True)
            nc.scalar.activation(outf[h][:], pso[:], AF.Copy)
            nc.vector.tensor_mul(outf[h][:], outf[h][:], rb[:])

        if DEBUG_OUTPUTS:
            for j in range(DT):
                nc.sync.dma_start(io["dbg_qfT"][j * P:(j + 1) * P, :], qf[j][:])
                nc.sync.dma_start(io["dbg_outfT"][j * P:(j + 1) * P, :],
                                  outf[j][:])
            for t in range(TT):
                nc.sync.dma_start(io["dbg_kf"][t * P:(t + 1) * P, :], kf[t][:])
                nc.sync.dma_start(io["dbg_vh"][t * P:(t + 1) * P, :], vh[t][:])
            nc.sync.dma_start(io["dbg_kv2"][:], kv2[:])

        # --- attn^T = Wo-stationary @ outf ; q_res = q + attn + bo ---
        qres = [live.tile([P, TOK], F32R, name=f"qres{j}") for j in range(DT)]
        for jh in range(2):
            pss = []
            for e in range(DT):
                wt = wst.tile([P, 4 * P], F32R, tag="wf", name="wt")
                nc.sync.dma_start(
                    wt[:], io["Wo"][e * P:(e + 1) * P,
                                    jh * 4 * P:(jh + 1) * 4 * P])
                for j in range(4):
                    if e == 0:
                        pss.append(pa())
                    nc.tensor.matmul(pss[j][:],
                                     lhsT=wt[:, j * P:(j + 1) * P],
                                     rhs=outf[e][:],
                                     start=(e == 0), stop=(e == DT - 1))
            for j in range(4):
                dj = jh * 4 + j
                qt = t2k(F32R)
                nc.sync.dma_start(qt[:], io["qT"][dj * P:(dj + 1) * P, :])
                nc.vector.scalar_tensor_tensor(
                    qres[dj][:], in0=pss[j][:], scalar=bo_c[:, dj:dj + 1],
                    in1=qt[:], op0=AluOpType.add, op1=AluOpType.add)

    if DEBUG_OUTPUTS:
        for d in range(DT):
            nc.sync.dma_start(io["dbg_qresT"][d * P:(d + 1) * P, :],
                              qres[d][:])

    # =================== router ===================
    xm = ln_cached(qres, LN["moe"], live, "xm")
    if DEBUG_OUTPUTS:
        for d in range(DT):
            nc.sync.dma_start(io["dbg_xmT"][d * P:(d + 1) * P, :], xm[d][:])

    psl = pc([E, TOK])
    for d in range(DT):
        nc.tensor.matmul(psl[:], lhsT=wr_sb[:, d * E:(d + 1) * E],
                         rhs=xm[d][:], start=(d == 0), stop=(d == DT - 1))
    logitsT = tmp.tile([E, TOK], F32, tag="lgt", bufs=1, name="logitsT")
    nc.vector.scalar_tensor_tensor(logitsT[:], in0=noise_sb[:], scalar=0.1,
                                   in1=psl[:], op0=AluOpType.mult,
                                   op1=AluOpType.add)
    nc.vector.tensor_scalar_add(logitsT[:], logitsT[:], br_c[:])

    comb = tmp.tile([P, TT * E], F32, tag="comb", bufs=1, name="comb")
    for t in range(TT):
        pst = pc([P, E])
        nc.tensor.transpose(pst[:], logitsT[:, t * P:(t + 1) * P],
                            ident[:E, :E])
        lg = tsm([P, E])
        nc.vector.tensor_copy(lg[:], pst[:])
        nmx = tsm([P, 1])
        nc.vector.reduce_max(nmx[:], lg[:], axis=AX.X, negate=True)
        ex = tsm([P, E])
        ssum = tsm([P, 1])
        nc.scalar.activation(ex[:], lg[:], AF.Exp, bias=nmx[:],
                             accum_out=ssum[:])
        rs = tsm([P, 1])
        nc.vector.reciprocal(rs[:], ssum[:])
        sc = tsm([P, E])
        nc.vector.tensor_scalar_mul(sc[:], ex[:], rs[:])
        m1v = tsm([P, 1])
        nc.vector.reduce_max(m1v[:], lg[:], axis=AX.X)
        m1 = tsm([P, E])
        nc.vector.tensor_scalar(m1[:], in0=lg[:], scalar1=m1v[:],
                                scalar2=None, op0=AluOpType.is_ge)
        lg2 = tsm([P, E])
        nc.vector.scalar_tensor_tensor(lg2[:], in0=m1[:], scalar=-1e30,
                                       in1=lg[:], op0=AluOpType.mult,
                                       op1=AluOpType.add)
        m2v = tsm([P, 1])
        nc.vector.reduce_max(m2v[:], lg2[:], axis=AX.X)
        msk = tsm([P, E])
        nc.vector.tensor_scalar(msk[:], in0=lg2[:], scalar1=m2v[:],
                                scalar2=None, op0=AluOpType.is_ge)
        nc.vector.tensor_add(msk[:], msk[:], m1[:])
        nc.vector.tensor_mul(comb[:, t * E:(t + 1) * E], msk[:], sc[:])

    if DEBUG_OUTPUTS:
        nc.sync.dma_start(io["dbg_logitsT"][:], logitsT[:])
        nc.sync.dma_start(io["dbg_comb"][:], comb[:])
    wT = live.tile([E, TOK], F32R, name="wT")
    for t in range(TT):
        pst = pc([E, P])
        nc.tensor.transpose(pst[:], comb[:, t * E:(t + 1) * E], ident[:])
        nc.scalar.activation(wT[:, t * P:(t + 1) * P], pst[:], AF.Copy)

    # =================== sparse top-2 dispatch ===================
    # Per (core, expert) capacity CAP. Slot s of expert e holds the
    # rank-s selected token (rank = # selected tokens before it).
    lmat = const.tile([P, P + 1], F32, name="lmat")
    nc.sync.dma_start(lmat[:], io["lmat"][:])
    iotac = const.tile([P, CAP], F32, name="iotac")
    nc.sync.dma_start(iotac[:], io["iotac"][:])
    ccols = const.tile([P, 2], F32, name="ccols")
    nc.sync.dma_start(ccols[:], io["ccols"][:])

    moep = ctx.enter_context(tc.tile_pool(name="moe", bufs=1))
    # ranks: rkw[:, :TOK] = masked rank (f32r), rkw[:, TOK:] = wT copy
    rkw = moep.tile([E, TOK], F32R, name="rkw")
    rkf = moep.tile([E, TOK], F32, name="rkf")
    mT = moep.tile([E, TOK], F32, name="mT")
    nc.vector.tensor_scalar(mT[:], in0=wT[:], scalar1=0.0, scalar2=None,
                            op0=AluOpType.is_gt)
    base = tmp.tile([E, 1], F32, tag="base", bufs=1, name="base")
    nc.vector.memset(base[:], 0.0)
    rank = tmp.tile([E, TOK], F32, tag="rank", bufs=1, name="rank")
    for t in range(TT):
        mk = tsm([P, E], F32)
        nc.vector.tensor_scalar(mk[:], in0=comb[:, t * E:(t + 1) * E],
                                scalar1=0.0, scalar2=None,
                                op0=AluOpType.is_gt)
        psr = pc([E, P + 1])
        nc.tensor.matmul(psr[:], lhsT=mk[:], rhs=lmat[:],
                         start=True, stop=True)
        nc.vector.tensor_scalar(rank[:, t * P:(t + 1) * P], in0=psr[:, :P],
                                scalar1=base[:], scalar2=None,
                                op0=AluOpType.add)
        nc.vector.tensor_add(base[:], base[:], psr[:, P:P + 1])
    # rk = (rank+1)*mask - 1  (unselected -> -1, never matches a slot)
    nc.vector.scalar_tensor_tensor(rkf[:], in0=rank[:], scalar=1.0,
                                   in1=mT[:], op0=AluOpType.add,
                                   op1=AluOpType.mult)
    nc.vector.tensor_scalar_add(rkf[:], rkf[:], -1.0)
    nc.vector.tensor_copy(rkw[:], rkf[:])

    # Gw[e][c, n] = w[e,n]/WSC iff slot c of e holds token n       (bf16)
    gw1 = [moep.tile([P, TOK], BF16, name=f"gw1_{e}") for e in range(E)]
    gw2 = [moep.tile([CAP - P, TOK], BF16, name=f"gw2_{e}") for e in range(E)]
    for e in range(E):
        psr2 = pa()
        nc.tensor.matmul(psr2[:], lhsT=esel[:, e * P:(e + 1) * P],
                         rhs=rkw[:], start=True, stop=True)
        psw = pa()
        nc.tensor.matmul(psw[:], lhsT=esel[:, e * P:(e + 1) * P],
                         rhs=wT[:], start=True, stop=True)
        ceq = t2k()
        nc.vector.tensor_scalar(ceq[:], in0=psr2[:],
                                scalar1=ccols[:, 0:1], scalar2=None,
                                op0=AluOpType.is_equal)
        nc.vector.scalar_tensor_tensor(gw1[e][:], in0=psw[:],
                                       scalar=1.0 / WSC, in1=ceq[:],
                                       op0=AluOpType.mult,
                                       op1=AluOpType.mult)
        ceq2 = tsm([CAP - P, TOK])
        nc.vector.tensor_scalar(ceq2[:], in0=psr2[:CAP - P, :],
                                scalar1=ccols[:CAP - P, 1:2], scalar2=None,
                                op0=AluOpType.is_equal)
        nc.vector.scalar_tensor_tensor(gw2[e][:], in0=psw[:CAP - P, :],
                                       scalar=1.0 / WSC, in1=ceq2[:],
                                       op0=AluOpType.mult,
                                       op1=AluOpType.mult)

    # gather: xb (token-major x, bf16) then XgT[d] = xb^T @ GT  (fp8)
    # GT[t][n, e*CAP+c] = 1 iff token (t,n) is slot c of expert e  (bf16)
    xgt = [moep.tile([P, E * CAP], FP8, name=f"xgt{d}") for d in range(DT)]
    with tc.tile_pool(name="gath", bufs=1) as gath:
        gts = [gath.tile([P, E * CAP], FP8, name=f"gt{t}")
               for t in range(TT)]
        for t in range(TT):
            pst = pc([P, E])
            nc.tensor.transpose(pst[:], rkf[:, t * P:(t + 1) * P],
                                ident[:E, :E])
            rkx = tsm([P, E])
            nc.vector.tensor_copy(rkx[:], pst[:])
            for e in range(E):
                nc.vector.tensor_scalar(gts[t][:, e * CAP:(e + 1) * CAP],
                                        in0=iotac[:], scalar1=rkx[:, e:e + 1],
                                        scalar2=None, op0=AluOpType.is_equal)
        xb = [gath.tile([P, D], FP8, name=f"xb{t}") for t in range(TT)]
        for t in range(TT):
            for g in range(2):
                psx = pa()
                for j in range(4):
                    d = g * 4 + j
                    nc.tensor.transpose(psx[:, j * P:(j + 1) * P],
                                        xm[d][:, t * P:(t + 1) * P],
                                        ident[:])
                nc.vector.tensor_copy(xb[t][:, g * 512:(g + 1) * 512],
                                      psx[:])
        NCH = (E * CAP + 511) // 512
        for d in range(DT):
            for ch in range(NCH):
                c0 = ch * 512
                c1 = min(c0 + 512, E * CAP)
                psg = psB.tile([P, c1 - c0], F32, space="PSUM", tag="b",
                               name="psg")
                for t in range(TT):
                    nc.tensor.matmul(psg[:],
                                     lhsT=xb[t][:, d * P:(d + 1) * P],
                                     rhs=gts[t][:, c0:c1],
                                     start=(t == 0), stop=(t == TT - 1))
                nc.vector.tensor_copy(xgt[d][:, c0:c1], psg[:])

    # =================== experts (fp8, sparse) ===================
    eo1 = [moep.tile([P, D], BF16, name=f"eo1_{e}") for e in range(E)]
    eo2 = [moep.tile([CAP - P, D], BF16, name=f"eo2_{e}") for e in range(E)]
    with tc.tile_pool(name="exp", bufs=1) as expp:
        for e in range(E):
            w1sb = expp.tile([P, DT * FF], FP8, tag="w1", name="w1sb")
            nc.sync.dma_start(w1sb[:], io["W1q"][e, :, :])
            w2sb = expp.tile([P, FT * D], FP8, tag="w2", name="w2sb")
            nc.sync.dma_start(w2sb[:], io["W2q"][e, :, :])
            ht = expp.tile([P, FT * CAP], FP8, tag="ht", name="ht")
            for m in range(FT):
                psh = psA.tile([P, CAP], F32, space="PSUM", tag="a",
                               name="psh")
                for d in range(DT):
                    nc.tensor.matmul(
                        psh[:],
                        lhsT=w1sb[:, d * FF + m * P:d * FF + (m + 1) * P],
                        rhs=xgt[d][:, e * CAP:(e + 1) * CAP],
                        start=(d == 0), stop=(d == DT - 1))
                nc.scalar.activation(ht[:, m * CAP:(m + 1) * CAP], psh[:],
                                     AF.Gelu,
                                     bias=b1_c[:, e * FT + m:e * FT + m + 1],
                                     scale=1.0 / WSC)
            for mo, msz, dst in ((0, P, eo1[e]), (P, CAP - P, eo2[e])):
                for n in range(2):
                    pse = psB.tile([msz, 512], F32, space="PSUM", tag="b",
                                   name="pse")
                    for k in range(FT):
                        nc.tensor.matmul(
                            pse[:],
                            lhsT=ht[:, k * CAP + mo:k * CAP + mo + msz],
                            rhs=w2sb[:, k * D + n * 512:k * D + (n + 1) * 512],
                            start=(k == 0), stop=(k == FT - 1))
                    nc.vector.tensor_copy(dst[:, n * 512:(n + 1) * 512],
                                          pse[:])

    # =================== scatter + tail: LN + residual ===================
    with tc.tile_pool(name="tail", bufs=1) as tail:
        u = []
        for d in range(DT):
            pss = pa()
            nc.tensor.matmul(pss[:], lhsT=b2_sb[:, d * P:(d + 1) * P],
                             rhs=wT[:], start=True, stop=False)
            for e in range(E):
                nc.tensor.matmul(pss[:], lhsT=eo1[e][:, d * P:(d + 1) * P],
                                 rhs=gw1[e][:], start=False, stop=False)
                nc.tensor.matmul(pss[:], lhsT=eo2[e][:, d * P:(d + 1) * P],
                                 rhs=gw2[e][:], start=False,
                                 stop=(e == E - 1))
            t = tail.tile([P, TOK], F32R, name=f"u{d}")
            nc.vector.tensor_add(t[:], pss[:], xm[d][:])
            u.append(t)
        if DEBUG_OUTPUTS:
            for d in range(DT):
                nc.sync.dma_start(io["dbg_moeT"][d * P:(d + 1) * P, :],
                                  u[d][:])

        def emit(d, a, c):
            mo = t2k()
            nc.vector.tensor_add(mo[:], a[:], c[:])
            yt = t2k()
            nc.vector.tensor_add(yt[:], qres[d][:], mo[:])
            nc.sync.dma_start(io["yT"][d * P:(d + 1) * P, :], yt[:])
        ln_t(lambda d, p: u[d][:], LN["out"], emit)


_NC_CACHE = None
_LAST_RES = None


def _get_nc():
    global _NC_CACHE
    if _NC_CACHE is None:
        _NC_CACHE = build_nc()
    return _NC_CACHE


def _bf16(a):
    """fast float32 -> bfloat16 round-to-nearest-even"""
    a = np.ascontiguousarray(a, dtype=np.float32)
    u = a.view(np.uint32)
    r = (u + 0x7FFF + ((u >> 16) & 1)) >> 16
    return np.ascontiguousarray(r.astype(np.uint16)).view(ml_dtypes.bfloat16)


def kernel(v, k, q, noise, g_v, b_v, g_k, b_k, g_q, b_q, g_moe, b_moe,
           g_out, b_out, Wq, Wk, Wv, Wo, bo, Wr, br, W1, b1, W2, b2, top_k):
    assert int(top_k) == 2
    nc = _get_nc()
    f32 = np.float32

    def col(x):  # [D] -> [128, DT] per-d-tile columns
        return np.ascontiguousarray(np.asarray(x, f32).reshape(-1, P).T)

    lncols = np.concatenate(
        [col(g_v), col(b_v), col(g_k), col(b_k), col(g_q), col(b_q),
         col(g_moe), col(b_moe), col(g_out), col(b_out)], axis=1)
    w1q = (np.asarray(W1, f32) * WSC).reshape(E, DT, P, FF)
    w1q = np.ascontiguousarray(w1q.transpose(0, 2, 1, 3).reshape(E, P, DT * FF))
    w2q = (np.asarray(W2, f32) * WSC).reshape(E, FT, P, D)
    w2q = np.ascontiguousarray(w2q.transpose(0, 2, 1, 3).reshape(E, P, FT * D))
    lmat = np.concatenate(
        [np.triu(np.ones((P, P), f32), k=1), np.ones((P, 1), f32)], axis=1)
    shared = {
        "Wq": np.asarray(Wq, f32), "Wk": np.asarray(Wk, f32),
        "Wv": np.asarray(Wv, f32), "Wo": np.asarray(Wo, f32),
        "Wr": np.asarray(Wr, f32),
        "W1q": w1q.astype(ml_dtypes.float8_e4m3),
        "W2q": w2q.astype(ml_dtypes.float8_e4m3),
        "lncols": np.ascontiguousarray(lncols), "bocol": col(bo),
        "brcol": np.ascontiguousarray(np.asarray(br, f32).reshape(E, 1)),
        "b1col": col(np.asarray(b1, f32).reshape(-1)),
        "b2mat": np.asarray(b2, f32),
        "esel": np.ascontiguousarray(
            np.repeat(np.eye(E, dtype=f32), P, axis=1).reshape(E, E * P)),
        "onesm": np.ones((P, P), f32),
        "lmat": lmat,
        "iotac": np.tile(np.arange(CAP, dtype=f32), (P, 1)),
        "ccols": np.stack([np.arange(P, dtype=f32),
                           P + np.arange(P, dtype=f32)], axis=1),
    }
    qf_ = np.asarray(q, f32).reshape(B * T, D)
    kf_ = np.asarray(k, f32).reshape(B * T, D)
    vf_ = np.asarray(v, f32).reshape(B * T, D)
    nf_ = np.asarray(noise, f32)
    in_maps = []
    for c in range(NCORES):
        sl = slice(c * TOK, (c + 1) * TOK)
        m = dict(shared)
        m["qT"] = np.ascontiguousarray(qf_[sl].T)
        m["kT"] = np.ascontiguousarray(kf_[sl].T)
        m["vT"] = np.ascontiguousarray(vf_[sl].T)
        m["noiseT"] = np.ascontiguousarray(nf_[sl].T)
        in_maps.append(m)

    res = run_bass_kernel_spmd(nc, in_maps, core_ids=list(range(NCORES)))
    global _LAST_RES
    _LAST_RES = res
    out = np.empty((B * T, D), f32)
    for c in range(NCORES):
        out[c * TOK:(c + 1) * TOK] = res.results[c]["yT"].T
    return out.reshape(B, T, D)


if __name__ == "__main__":
    build_nc()
    print("build ok")

